# revision 16
# baseline (speedup 1.0000x reference)
"""Trainium2 Bass kernel for a BERT-style self-attention block (B=2, S=4096,
H=768, NH=12) sharded over 8 NeuronCores.

Sharding: data-parallel over batch (2) x query-block parallel (4) = 8 cores.
Each core computes K/V for the full sequence of its batch (replicated within
the 4-core group) and a disjoint 1024-query slice of the output, so no
collectives are needed and the LayerNorm epilogue is fully local.

Per-core dataflow (all matmuls bf16, fp32 accumulate):
  xT [H,S]  --PE-->  K^T [H,S], V [S,H] (+bias), Q^T [H,SQ] (pre-scaled 1/8)
  scores^T [skey,q] = K_h^T.T @ Q_h^T   (two heads row-packed per PE pass)
  P^T = exp(scores^T + mask[skey])      (ScalarE, mask is per-partition bias)
  ctx_u^T [65,q] = [V_h | 1].T @ P^T    (ones column -> row 64 = softmax denom)
  ctx^T = ctx_u^T * recip(denom)        (DMA partition-broadcast of recip row)
  out[s,:] = LN(ctx @ o_w.T + o_b + x)  (residual folded into xres on host)
"""

import numpy as np
import ml_dtypes

B, S, H, NH = 2, 4096, 768, 12
HD = H // NH  # 64
NCORES = 8
SQ = S // 4  # 1024 queries per core
LN_EPS = 1e-12

_BUILD_CACHE = {}


def build(S_=S, SQ_=SQ, stage="full"):
    """Build (and cache) the single-core Bass program. All 8 cores run this
    same program on different inputs. stage in {"proj","attn","full"} for
    debugging bisection."""
    key = (S_, SQ_, stage)
    if key in _BUILD_CACHE:
        return _BUILD_CACHE[key]

    import concourse.mybir as mybir
    import concourse.tile as tile
    from concourse import bacc

    dt = mybir.dt
    f32, bf16 = dt.float32, dt.bfloat16
    AF = mybir.ActivationFunctionType
    OP = mybir.AluOpType

    EC = H // 128           # 6 contraction chunks over H
    OC = H // 128           # 6 output-channel chunks over H
    NSC512 = S_ // 512      # 512-wide s chunks (projection phase)
    NST = S_ // 128         # 128-wide s tiles (attention skey chunks)
    QT = min(512, SQ_)      # q tile for attention
    NQT = SQ_ // QT
    QCH = min(512, SQ_)     # q chunk for Q projection
    NQCH = SQ_ // QCH
    NP2 = NH // 2           # head pairs
    NSTQ = SQ_ // 128       # output s tiles

    nc = bacc.Bacc("TRN2", target_bir_lowering=False, debug=False)

    xT = nc.dram_tensor("xT", [H, S_], bf16, kind="ExternalInput")
    xTq = nc.dram_tensor("xTq", [H, SQ_], bf16, kind="ExternalInput")
    wqT = nc.dram_tensor("wqT", [H, H], bf16, kind="ExternalInput")
    wkT = nc.dram_tensor("wkT", [H, H], bf16, kind="ExternalInput")
    wvT = nc.dram_tensor("wvT", [H, H], bf16, kind="ExternalInput")
    woT = nc.dram_tensor("woT", [H, H], bf16, kind="ExternalInput")
    qb = nc.dram_tensor("qb", [H], f32, kind="ExternalInput")
    kb = nc.dram_tensor("kb", [H], f32, kind="ExternalInput")
    vb = nc.dram_tensor("vb", [H], f32, kind="ExternalInput")
    mask = nc.dram_tensor("mask", [S_], f32, kind="ExternalInput")
    xres = nc.dram_tensor("xres", [SQ_, H], f32, kind="ExternalInput")
    lng = nc.dram_tensor("lng", [H], f32, kind="ExternalInput")
    lnb = nc.dram_tensor("lnb", [H], f32, kind="ExternalInput")
    out = nc.dram_tensor("out", [SQ_, H], f32, kind="ExternalOutput")

    with tile.TileContext(nc) as tc:
        with tc.tile_pool(name="persist", bufs=1) as pp:
            kt_sb = pp.tile([128, OC, S_], bf16, tag="kt")
            qt_sb = pp.tile([128, OC, SQ_], bf16, tag="qt")
            v_sb = pp.tile([128, NST, NH, HD + 1], bf16, tag="v")
            ctxT = pp.tile([128, EC, SQ_], bf16, tag="ctxT")
            wo_sb = pp.tile([128, EC, H], bf16, tag="wo")
            qb_sb = pp.tile([128, OC], f32, tag="qb")
            kb_sb = pp.tile([128, OC], f32, tag="kb")
            mask_sb = pp.tile([128, NST], f32, tag="mask")
            vb_bc = pp.tile([128, H], f32, tag="vbbc")
            lng_bc = pp.tile([128, H], f32, tag="lngbc")
            lnb_bc = pp.tile([128, H], f32, tag="lnbbc")

            nc.sync.dma_start(wo_sb[:], woT.rearrange("(c p) o -> p c o", p=128))
            nc.sync.dma_start(qb_sb[:], qb.rearrange("(c p) -> p c", p=128))
            nc.sync.dma_start(kb_sb[:], kb.rearrange("(c p) -> p c", p=128))
            nc.sync.dma_start(mask_sb[:], mask.rearrange("(c p) -> p c", p=128))
            nc.sync.dma_start(vb_bc[:], vb[None, :].to_broadcast((128, H)))
            nc.sync.dma_start(lng_bc[:], lng[None, :].to_broadcast((128, H)))
            nc.sync.dma_start(lnb_bc[:], lnb[None, :].to_broadcast((128, H)))
            nc.vector.memset(v_sb[:, :, :, HD : HD + 1], 1.0)

            # ---------------- phase 1: Q/K/V projections ----------------
            with tc.tile_pool(name="ph1w", bufs=1) as wp, \
                 tc.tile_pool(name="xtp", bufs=3) as xtp, \
                 tc.tile_pool(name="ps1", bufs=2, space="PSUM") as ps1:
                wq_sb = wp.tile([128, EC, H], bf16, tag="wq")
                wk_sb = wp.tile([128, EC, H], bf16, tag="wk")
                wv_sb = wp.tile([128, EC, H], bf16, tag="wv")
                nc.sync.dma_start(wq_sb[:], wqT.rearrange("(c p) o -> p c o", p=128))
                nc.sync.dma_start(wk_sb[:], wkT.rearrange("(c p) o -> p c o", p=128))
                nc.sync.dma_start(wv_sb[:], wvT.rearrange("(c p) o -> p c o", p=128))

                xT_r = xT.rearrange("(c p) s -> p c s", p=128)
                pv3 = None
                for sc in range(NSC512):
                    xt = xtp.tile([128, EC, 512], bf16, tag="xt")
                    nc.sync.dma_start(xt[:], xT_r[:, :, sc * 512 : (sc + 1) * 512])
                    # K^T o-tiles
                    for oc in range(OC):
                        pk = ps1.tile([128, 512], f32, tag="pk")
                        for ec in range(EC):
                            nc.tensor.matmul(
                                pk[:],
                                wk_sb[:, ec, oc * 128 : (oc + 1) * 128],
                                xt[:, ec],
                                start=(ec == 0),
                                stop=(ec == EC - 1),
                            )
                        nc.vector.tensor_scalar_add(
                            kt_sb[:, oc, sc * 512 : (sc + 1) * 512],
                            pk[:],
                            kb_sb[:, oc : oc + 1],
                        )
                    # V s-tiles (natural [s, d] layout)
                    for t4 in range(4):
                        st = sc * 4 + t4
                        pv = ps1.tile([128, H], f32, tag="pv")
                        for ec in range(EC):
                            xs = xt[:, ec, t4 * 128 : (t4 + 1) * 128]
                            nc.tensor.matmul(
                                pv[:, 0:512], xs, wv_sb[:, ec, 0:512],
                                start=(ec == 0), stop=(ec == EC - 1),
                            )
                            nc.tensor.matmul(
                                pv[:, 512:H], xs, wv_sb[:, ec, 512:H],
                                start=(ec == 0), stop=(ec == EC - 1),
                            )
                        nc.vector.tensor_tensor(
                            v_sb[:, st, :, 0:HD],
                            pv.rearrange("p (h d) -> p h d", d=HD),
                            vb_bc.rearrange("p (h d) -> p h d", d=HD),
                            OP.add,
                        )
                # Q^T from the query-window slice
                xTq_r = xTq.rearrange("(c p) s -> p c s", p=128)
                for qc in range(NQCH):
                    xtq = xtp.tile([128, EC, 512], bf16, tag="xt")
                    nc.sync.dma_start(
                        xtq[:, :, 0:QCH], xTq_r[:, :, qc * QCH : (qc + 1) * QCH]
                    )
                    for oc in range(OC):
                        pq = ps1.tile([128, 512], f32, tag="pk")
                        for ec in range(EC):
                            nc.tensor.matmul(
                                pq[:, 0:QCH],
                                wq_sb[:, ec, oc * 128 : (oc + 1) * 128],
                                xtq[:, ec, 0:QCH],
                                start=(ec == 0),
                                stop=(ec == EC - 1),
                            )
                        nc.vector.tensor_scalar_add(
                            qt_sb[:, oc, qc * QCH : (qc + 1) * QCH],
                            pq[:, 0:QCH],
                            qb_sb[:, oc : oc + 1],
                        )

            if stage == "proj":
                nc.gpsimd.dma_start(out[0:128, :], qt_sb[:, :, 0:128])
                nc.gpsimd.dma_start(out[128:256, :], kt_sb[:, :, 0:128])
            # ---------------- phase 2: attention ----------------
            do_ctx = stage not in ("attn1",)
            do_norm = stage not in ("attn1", "attn2")
            if stage != "proj":
              with tc.tile_pool(name="ptp", bufs=6) as ptp, \
                 tc.tile_pool(name="rdp", bufs=3) as rdp, \
                 tc.tile_pool(name="rddr", bufs=3, space="DRAM") as rddr, \
                 tc.tile_pool(name="ps2s", bufs=2, space="PSUM") as ps2s, \
                 tc.tile_pool(name="ps2c", bufs=4, space="PSUM") as ps2c:
                for p2 in range(NP2):
                    for qt in range(NQT):
                        q0 = qt * QT
                        cx = [
                            ps2c.tile([HD + 1, QT], f32, tag="ctx", name=f"ctx{i}")
                            for i in range(2)
                        ]
                        for sc in range(NST):
                            # each head's scores in its OWN psum bank: two
                            # concurrent row-packed matmuls into one bank is a
                            # hardware fault
                            sp = ps2s.tile([128, 2, 512], f32, tag="sc")
                            nc.tensor.matmul(
                                sp[:, 0, 0:QT],
                                kt_sb[0:64, p2, sc * 128 : (sc + 1) * 128],
                                qt_sb[0:64, p2, q0 : q0 + QT],
                                start=True, stop=True,
                            )
                            nc.tensor.matmul(
                                sp[:, 1, 0:QT],
                                kt_sb[64:128, p2, sc * 128 : (sc + 1) * 128],
                                qt_sb[64:128, p2, q0 : q0 + QT],
                                start=True, stop=True,
                            )
                            pt = ptp.tile([128, 2, QT], bf16, tag="pt")
                            nc.scalar.activation(
                                pt[:], sp[:, :, 0:QT], AF.Exp,
                                bias=mask_sb[:, sc : sc + 1], scale=1.0,
                            )
                            if stage == "attn1":
                                nc.gpsimd.dma_start(
                                    out[0:128, 0:2 * QT],
                                    pt.rearrange("p a q -> p (a q)"),
                                )
                            if do_ctx:
                                nc.tensor.matmul(
                                    cx[0][:], v_sb[:, sc, 2 * p2, :], pt[:, 0, :],
                                    start=(sc == 0), stop=(sc == NST - 1),
                                )
                                nc.tensor.matmul(
                                    cx[1][:], v_sb[:, sc, 2 * p2 + 1, :],
                                    pt[:, 1, :],
                                    start=(sc == 0), stop=(sc == NST - 1),
                                )
                        if stage == "attn2":
                            dbg = rdp.tile([65, QT], f32, tag="dbg")
                            nc.vector.tensor_copy(dbg[:], cx[0][:])
                            nc.gpsimd.dma_start(out[0:65, 0:QT], dbg[:])
                        for i01 in range(2) if do_norm else []:
                            h = 2 * p2 + i01
                            rd = rdp.tile([1, QT], f32, tag="rd")
                            nc.vector.reciprocal(rd[:], cx[i01][HD : HD + 1, :])
                            rdd = rddr.tile([1, QT], f32, tag="rdd")
                            nc.sync.dma_start(rdd[:], rd[:])
                            rdb = rdp.tile([64, QT], f32, tag="rdb")
                            nc.sync.dma_start(
                                rdb[:], rdd[:].to_broadcast((64, QT))
                            )
                            nc.vector.tensor_tensor(
                                ctxT[
                                    (h % 2) * 64 : (h % 2) * 64 + 64,
                                    h // 2,
                                    q0 : q0 + QT,
                                ],
                                cx[i01][0:HD, :],
                                rdb[:],
                                OP.mult,
                            )

            if stage == "attn":
                nc.gpsimd.dma_start(out[0:128, :], ctxT[:, :, 0:128])
            # ---------------- tail: O-projection + residual + LN ----------------
            if stage == "full":
              with tc.tile_pool(name="tail", bufs=2) as tp, \
                 tc.tile_pool(name="ys", bufs=NSTQ) as yp, \
                 tc.tile_pool(name="ps3", bufs=2, space="PSUM") as ps3:
                eps_ap = tp.tile([128, 1], f32, tag="eps")
                nc.vector.memset(eps_ap[:], float(LN_EPS))
                ss_all = tp.tile([128, NSTQ], f32, tag="ss")
                negmu_all = tp.tile([128, NSTQ], f32, tag="negmu")
                std_all = tp.tile([128, NSTQ], f32, tag="std")
                rstd_all = tp.tile([128, NSTQ], f32, tag="rstd")
                ys = []
                for st in range(NSTQ):
                    po = ps3.tile([128, H], f32, tag="po")
                    for dc in range(EC):
                        lh = ctxT[:, dc, st * 128 : (st + 1) * 128]
                        nc.tensor.matmul(
                            po[:, 0:512], lh, wo_sb[:, dc, 0:512],
                            start=(dc == 0), stop=(dc == EC - 1),
                        )
                        nc.tensor.matmul(
                            po[:, 512:H], lh, wo_sb[:, dc, 512:H],
                            start=(dc == 0), stop=(dc == EC - 1),
                        )
                    xr = tp.tile([128, H], f32, tag="xr")
                    nc.sync.dma_start(xr[:], xres[st * 128 : (st + 1) * 128, :])
                    y = yp.tile([128, H], f32, tag="y")
                    ysum = tp.tile([128, 1], f32, tag="ysum")
                    nc.vector.tensor_tensor(y[:], po[:], xr[:], OP.add)
                    nc.vector.reduce_sum(ysum[:], y[:], axis=mybir.AxisListType.X)
                    nc.vector.tensor_scalar_mul(
                        negmu_all[:, st : st + 1], ysum[:], -1.0 / H
                    )
                    sq = tp.tile([128, H], f32, tag="sq")
                    nc.scalar.activation(
                        sq[:], y[:], AF.Square,
                        bias=negmu_all[:, st : st + 1], scale=1.0,
                        accum_out=ss_all[:, st : st + 1],
                    )
                    ys.append(y)
                nc.scalar.activation(
                    std_all[:], ss_all[:], AF.Sqrt, bias=eps_ap[:, 0:1], scale=1.0 / H
                )
                nc.vector.reciprocal(rstd_all[:], std_all[:])
                for st in range(NSTQ):
                    t1 = tp.tile([128, H], f32, tag="t1")
                    nc.vector.tensor_scalar(
                        t1[:], ys[st][:],
                        negmu_all[:, st : st + 1], rstd_all[:, st : st + 1],
                        OP.add, OP.mult,
                    )
                    t2 = tp.tile([128, H], f32, tag="t2")
                    nc.vector.tensor_tensor(t2[:], t1[:], lng_bc[:], OP.mult)
                    ot = tp.tile([128, H], f32, tag="ot")
                    nc.vector.tensor_tensor(ot[:], t2[:], lnb_bc[:], OP.add)
                    nc.sync.dma_start(out[st * 128 : (st + 1) * 128, :], ot[:])

    nc.compile()
    _BUILD_CACHE[key] = nc
    return nc


def make_in_maps(inputs, S_=S, SQ_=SQ):
    """Host-side sharding: slice/transpose/cast the full inputs into the 8
    per-core input maps."""
    bf16 = ml_dtypes.bfloat16
    hs = np.ascontiguousarray(np.asarray(inputs["hidden_states"], np.float32))
    am = np.asarray(inputs["attention_mask"], np.float32)
    q_w = np.asarray(inputs["q_w"], np.float32)
    k_w = np.asarray(inputs["k_w"], np.float32)
    v_w = np.asarray(inputs["v_w"], np.float32)
    o_w = np.asarray(inputs["o_w"], np.float32)
    q_b = np.asarray(inputs["q_b"], np.float32)
    k_b = np.asarray(inputs["k_b"], np.float32)
    v_b = np.asarray(inputs["v_b"], np.float32)
    o_b = np.asarray(inputs["o_b"], np.float32)
    ln_g = np.asarray(inputs["ln_g"], np.float32)
    ln_b = np.asarray(inputs["ln_b"], np.float32)

    scale = 1.0 / np.sqrt(HD)
    wqT = np.ascontiguousarray((q_w.T * scale).astype(bf16))
    wkT = np.ascontiguousarray(k_w.T.astype(bf16))
    wvT = np.ascontiguousarray(v_w.T.astype(bf16))
    woT = np.ascontiguousarray(o_w.T.astype(bf16))
    qbs = (q_b * scale).astype(np.float32)

    nb = hs.shape[0]
    xT_full = [np.ascontiguousarray(hs[b].T.astype(bf16)) for b in range(nb)]
    groups = NCORES // nb  # query-parallel cores per batch

    in_maps = []
    for c in range(NCORES):
        b, j = c // groups, c % groups
        sl = slice(j * SQ_, (j + 1) * SQ_)
        in_maps.append(
            {
                "xT": xT_full[b],
                "xTq": np.ascontiguousarray(xT_full[b][:, sl]),
                "wqT": wqT, "wkT": wkT, "wvT": wvT, "woT": woT,
                "qb": qbs, "kb": k_b, "vb": v_b,
                "mask": np.ascontiguousarray(am[b, 0, 0]),
                "xres": np.ascontiguousarray(hs[b, sl] + o_b[None, :]),
                "lng": ln_g, "lnb": ln_b,
            }
        )
    return in_maps


def run_cores(inputs, trace=False, **kwargs):
    from concourse.bass_utils import run_bass_kernel_spmd

    nc = build()
    in_maps = make_in_maps(inputs)
    res = run_bass_kernel_spmd(
        nc, in_maps, core_ids=list(range(NCORES)), trace=trace, **kwargs
    )
    nb = np.asarray(inputs["hidden_states"]).shape[0]
    groups = NCORES // nb
    out = np.empty((nb, S, H), np.float32)
    for c in range(NCORES):
        b, j = c // groups, c % groups
        out[b, j * SQ : (j + 1) * SQ] = res.results[c]["out"]
    return out, res


def kernel(**inputs):
    out, _ = run_cores(inputs, trace=False)
    return out


# revision 19
# speedup vs baseline: 109.6352x; 109.6352x over previous
"""Trainium2 Bass kernel for a BERT-style self-attention block (B=2, S=4096,
H=768, NH=12) sharded over 8 NeuronCores.

Sharding: data-parallel over batch (2) x query-block parallel (4) = 8 cores.
Each core computes K/V for the full sequence of its batch (replicated within
the 4-core group) and a disjoint 1024-query slice of the output, so no
collectives are needed and the LayerNorm epilogue is fully local.

Per-core dataflow (all matmuls bf16, fp32 accumulate):
  xT [H,S]  --PE-->  K^T [H,S], V [S,H] (+bias), Q^T [H,SQ] (pre-scaled 1/8)
  scores^T [skey,q] = K_h^T.T @ Q_h^T   (two heads row-packed per PE pass,
                                         each into its OWN psum bank)
  P^T = exp(scores^T + mask[skey])      (ScalarE, mask is per-partition bias)
  ctx_u^T [65,q] = [V_h | 1].T @ P^T    (ones column -> row 64 = softmax denom)
  ctx^T = ctx_u^T * recip(denom)        (partition-broadcast via DRAM bounce)
  out[s,:] = LN(ctx @ o_w.T + o_b + x)  (residual folded into xres on host)

All compute phases share one 8-bank PSUM budget and live in concurrently-open
pools so the Tile scheduler can overlap the projections with early attention
pairs; the bulk SBUF pool (K^T/V/Q^T/weights) closes before the LN tail opens
so the tail working set fits under the 208KB/partition cap.
"""

import numpy as np
import ml_dtypes

B, S, H, NH = 2, 4096, 768, 12
HD = H // NH  # 64
NCORES = 8
SQ = S // 4  # 1024 queries per core
LN_EPS = 1e-12

_BUILD_CACHE = {}


def build(S_=S, SQ_=SQ, stage="full", repeat=1):
    key = (S_, SQ_, stage, repeat)
    if key in _BUILD_CACHE:
        return _BUILD_CACHE[key]

    import concourse.mybir as mybir
    import concourse.tile as tile
    from concourse import bacc

    dt = mybir.dt
    f32, bf16 = dt.float32, dt.bfloat16
    AF = mybir.ActivationFunctionType
    OP = mybir.AluOpType

    EC = H // 128           # 6 contraction chunks over H
    OC = H // 128           # 6 output-channel chunks over H
    NSC512 = S_ // 512      # 512-wide s chunks (projection phase)
    NST = S_ // 128         # 128-wide s tiles (attention skey chunks)
    QT = min(512, SQ_)      # q tile for attention
    NQT = SQ_ // QT
    QCH = min(512, SQ_)     # q chunk for Q projection
    NQCH = SQ_ // QCH
    NP2 = NH // 2           # head pairs
    NSTQ = SQ_ // 128       # output s tiles

    nc = bacc.Bacc("TRN2", target_bir_lowering=False, debug=False)

    xT = nc.dram_tensor("xT", [H, S_], bf16, kind="ExternalInput")
    xTq = nc.dram_tensor("xTq", [H, SQ_], bf16, kind="ExternalInput")
    wqT = nc.dram_tensor("wqT", [H, H], bf16, kind="ExternalInput")
    wkT = nc.dram_tensor("wkT", [H, H], bf16, kind="ExternalInput")
    wvT = nc.dram_tensor("wvT", [H, H], bf16, kind="ExternalInput")
    woT = nc.dram_tensor("woT", [H, H], bf16, kind="ExternalInput")
    qb = nc.dram_tensor("qb", [H], f32, kind="ExternalInput")
    kb = nc.dram_tensor("kb", [H], f32, kind="ExternalInput")
    vb = nc.dram_tensor("vb", [H], f32, kind="ExternalInput")
    mask = nc.dram_tensor("mask", [S_], f32, kind="ExternalInput")
    xres = nc.dram_tensor("xres", [SQ_, H], f32, kind="ExternalInput")
    lng = nc.dram_tensor("lng", [H], f32, kind="ExternalInput")
    lnb = nc.dram_tensor("lnb", [H], f32, kind="ExternalInput")
    out = nc.dram_tensor("out", [SQ_, H], f32, kind="ExternalOutput")

    def emit_qkv(nc, pools):
        (xtp, psA, psK, wq_sb, wk_sb, wv_sb, kt_sb, qt_sb, v_sb) = pools
        # Q projection first: small, unblocks attention early
        xTq_r = xTq.rearrange("(c p) s -> p c s", p=128)
        for qc in range(NQCH):
            xtq = xtp.tile([128, EC, 512], bf16, tag="xt")
            nc.sync.dma_start(
                xtq[:, :, 0:QCH], xTq_r[:, :, qc * QCH : (qc + 1) * QCH]
            )
            for oc in range(OC):
                pq = psK.tile([128, 512], f32, tag="pk")
                for ec in range(EC):
                    nc.tensor.matmul(
                        pq[:, 0:QCH],
                        wq_sb[:, ec, oc * 128 : (oc + 1) * 128],
                        xtq[:, ec, 0:QCH],
                        start=(ec == 0), stop=(ec == EC - 1),
                    )
                nc.vector.tensor_scalar_add(
                    qt_sb[:, oc, qc * QCH : (qc + 1) * QCH],
                    pq[:, 0:QCH],
                    qb_sb[:, oc : oc + 1],
                )
        # K^T and V over the full sequence
        xT_r = xT.rearrange("(c p) s -> p c s", p=128)
        for sc in range(NSC512):
            xt = xtp.tile([128, EC, 512], bf16, tag="xt")
            nc.sync.dma_start(xt[:], xT_r[:, :, sc * 512 : (sc + 1) * 512])
            for oc in range(OC):
                pk = psK.tile([128, 512], f32, tag="pk")
                for ec in range(EC):
                    nc.tensor.matmul(
                        pk[:],
                        wk_sb[:, ec, oc * 128 : (oc + 1) * 128],
                        xt[:, ec],
                        start=(ec == 0), stop=(ec == EC - 1),
                    )
                nc.vector.tensor_scalar_add(
                    kt_sb[:, oc, sc * 512 : (sc + 1) * 512],
                    pk[:],
                    kb_sb[:, oc : oc + 1],
                )
            for t4 in range(4):
                st = sc * 4 + t4
                pva = psK.tile([128, 512], f32, tag="pk")
                pvb = psK.tile([128, 512], f32, tag="pk")
                for ec in range(EC):
                    xs = xt[:, ec, t4 * 128 : (t4 + 1) * 128]
                    nc.tensor.matmul(
                        pva[:], xs, wv_sb[:, ec, 0:512],
                        start=(ec == 0), stop=(ec == EC - 1),
                    )
                    nc.tensor.matmul(
                        pvb[:, 0 : H - 512], xs, wv_sb[:, ec, 512:H],
                        start=(ec == 0), stop=(ec == EC - 1),
                    )
                nc.vector.tensor_tensor(
                    v_sb[:, st, 0:8, 0:HD],
                    pva.rearrange("p (h d) -> p h d", d=HD),
                    vb_bc[:, 0:512].rearrange("p (h d) -> p h d", d=HD),
                    OP.add,
                )
                nc.vector.tensor_tensor(
                    v_sb[:, st, 8:NH, 0:HD],
                    pvb[:, 0 : H - 512].rearrange("p (h d) -> p h d", d=HD),
                    vb_bc[:, 512:H].rearrange("p (h d) -> p h d", d=HD),
                    OP.add,
                )

    def emit_attention(nc, pools):
        (ptp, rdp, rddr, psA, psC, kt_sb, qt_sb, v_sb) = pools
        do_ctx = stage not in ("attn1",)
        do_norm = stage not in ("attn1", "attn2")
        for qt in range(NQT):
            q0 = qt * QT
            for p2 in range(NP2):
                cx = [
                    psC.tile([HD + 1, QT], f32, tag="ctx", name=f"ctx{i}")
                    for i in range(2)
                ]
                for sc in range(NST):
                    sp = psA.tile([128, 1024], f32, tag="big")
                    nc.tensor.matmul(
                        sp[:, 0:QT],
                        kt_sb[0:64, p2, sc * 128 : (sc + 1) * 128],
                        qt_sb[0:64, p2, q0 : q0 + QT],
                        start=True, stop=True,
                    )
                    nc.tensor.matmul(
                        sp[:, 512 : 512 + QT],
                        kt_sb[64:128, p2, sc * 128 : (sc + 1) * 128],
                        qt_sb[64:128, p2, q0 : q0 + QT],
                        start=True, stop=True,
                    )
                    pt = ptp.tile([128, 2, QT], bf16, tag="pt")
                    nc.scalar.activation(
                        pt[:],
                        sp.rearrange("p (a q) -> p a q", a=2)[:, :, 0:QT],
                        AF.Exp,
                        bias=mask_sb[:, sc : sc + 1], scale=1.0,
                    )
                    if stage == "attn1":
                        nc.gpsimd.dma_start(
                            out[0:128, 0 : 2 * QT],
                            pt.rearrange("p a q -> p (a q)"),
                        )
                    if do_ctx:
                        nc.tensor.matmul(
                            cx[0][:], v_sb[:, sc, 2 * p2, :], pt[:, 0, :],
                            start=(sc == 0), stop=(sc == NST - 1),
                        )
                        nc.tensor.matmul(
                            cx[1][:], v_sb[:, sc, 2 * p2 + 1, :], pt[:, 1, :],
                            start=(sc == 0), stop=(sc == NST - 1),
                        )
                if stage == "attn2":
                    dbg = rdp.tile([65, QT], f32, tag="dbg")
                    nc.vector.tensor_copy(dbg[:], cx[0][:])
                    nc.gpsimd.dma_start(out[0:65, 0:QT], dbg[:])
                for i01 in range(2) if do_norm else []:
                    h = 2 * p2 + i01
                    rd = rdp.tile([1, QT], f32, tag="rd")
                    nc.vector.reciprocal(rd[:], cx[i01][HD : HD + 1, :])
                    rdd = rddr.tile([1, QT], f32, tag="rdd")
                    nc.sync.dma_start(rdd[:], rd[:])
                    rdb = rdp.tile([64, QT], f32, tag="rdb")
                    nc.sync.dma_start(rdb[:], rdd[:].to_broadcast((64, QT)))
                    nc.vector.tensor_tensor(
                        ctxT[
                            (h % 2) * 64 : (h % 2) * 64 + 64,
                            h // 2,
                            q0 : q0 + QT,
                        ],
                        cx[i01][0:HD, :],
                        rdb[:],
                        OP.mult,
                    )

    def emit_tail(nc, tc):
        with tc.tile_pool(name="tailc", bufs=1) as tpc, \
             tc.tile_pool(name="tailw", bufs=3) as tpw, \
             tc.tile_pool(name="ys", bufs=NSTQ) as yp, \
             tc.tile_pool(name="ps3", bufs=2, space="PSUM") as ps3:
            wo_sb = tpc.tile([128, EC, H], bf16, tag="wo")
            lng_bc = tpc.tile([128, H], f32, tag="lngbc")
            lnb_bc = tpc.tile([128, H], f32, tag="lnbbc")
            eps_ap = tpc.tile([128, 1], f32, tag="eps")
            ss_all = tpc.tile([128, NSTQ], f32, tag="ss")
            negmu_all = tpc.tile([128, NSTQ], f32, tag="negmu")
            std_all = tpc.tile([128, NSTQ], f32, tag="std")
            rstd_all = tpc.tile([128, NSTQ], f32, tag="rstd")
            nc.sync.dma_start(wo_sb[:], woT.rearrange("(c p) o -> p c o", p=128))
            nc.sync.dma_start(lng_bc[:], lng[None, :].to_broadcast((128, H)))
            nc.sync.dma_start(lnb_bc[:], lnb[None, :].to_broadcast((128, H)))
            nc.vector.memset(eps_ap[:], float(LN_EPS))
            ys = []
            for st in range(NSTQ):
                po = ps3.tile([128, H], f32, tag="po")
                for dc in range(EC):
                    lh = ctxT[:, dc, st * 128 : (st + 1) * 128]
                    nc.tensor.matmul(
                        po[:, 0:512], lh, wo_sb[:, dc, 0:512],
                        start=(dc == 0), stop=(dc == EC - 1),
                    )
                    nc.tensor.matmul(
                        po[:, 512:H], lh, wo_sb[:, dc, 512:H],
                        start=(dc == 0), stop=(dc == EC - 1),
                    )
                xr = tpw.tile([128, H], f32, tag="xr")
                nc.sync.dma_start(xr[:], xres[st * 128 : (st + 1) * 128, :])
                y = yp.tile([128, H], f32, tag="y")
                ysum = tpw.tile([128, 1], f32, tag="ysum")
                nc.vector.tensor_tensor(y[:], po[:], xr[:], OP.add)
                nc.vector.reduce_sum(ysum[:], y[:], axis=mybir.AxisListType.X)
                nc.vector.tensor_scalar_mul(
                    negmu_all[:, st : st + 1], ysum[:], -1.0 / H
                )
                sq = tpw.tile([128, H], f32, tag="scratch")
                nc.scalar.activation(
                    sq[:], y[:], AF.Square,
                    bias=negmu_all[:, st : st + 1], scale=1.0,
                    accum_out=ss_all[:, st : st + 1],
                )
                ys.append(y)
            nc.scalar.activation(
                std_all[:], ss_all[:], AF.Sqrt,
                bias=eps_ap[:, 0:1], scale=1.0 / H,
            )
            nc.vector.reciprocal(rstd_all[:], std_all[:])
            for st in range(NSTQ):
                t1 = tpw.tile([128, H], f32, tag="scratch")
                nc.vector.tensor_scalar(
                    t1[:], ys[st][:],
                    negmu_all[:, st : st + 1], rstd_all[:, st : st + 1],
                    OP.add, OP.mult,
                )
                t2 = tpw.tile([128, H], f32, tag="scratch")
                nc.vector.tensor_tensor(t2[:], t1[:], lng_bc[:], OP.mult)
                ot = tpw.tile([128, H], f32, tag="scratch")
                nc.vector.tensor_tensor(ot[:], t2[:], lnb_bc[:], OP.add)
                nc.sync.dma_start(out[st * 128 : (st + 1) * 128, :], ot[:])

    with tile.TileContext(nc) as tc:
        with tc.tile_pool(name="persist", bufs=1) as pp:
            ctxT = pp.tile([128, EC, SQ_], bf16, tag="ctxT")
            qb_sb = pp.tile([128, OC], f32, tag="qb")
            kb_sb = pp.tile([128, OC], f32, tag="kb")
            mask_sb = pp.tile([128, NST], f32, tag="mask")
            vb_bc = pp.tile([128, H], f32, tag="vbbc")
            nc.sync.dma_start(qb_sb[:], qb.rearrange("(c p) -> p c", p=128))
            nc.sync.dma_start(kb_sb[:], kb.rearrange("(c p) -> p c", p=128))
            nc.sync.dma_start(mask_sb[:], mask.rearrange("(c p) -> p c", p=128))
            nc.sync.dma_start(vb_bc[:], vb[None, :].to_broadcast((128, H)))

            for rep_ in range(repeat):
                with tc.tile_pool(name="bulk", bufs=1) as bulk:
                    kt_sb = bulk.tile([128, OC, S_], bf16, tag="kt")
                    qt_sb = bulk.tile([128, OC, SQ_], bf16, tag="qt")
                    v_sb = bulk.tile([128, NST, NH, HD + 1], bf16, tag="v")
                    wq_sb = bulk.tile([128, EC, H], bf16, tag="wq")
                    wk_sb = bulk.tile([128, EC, H], bf16, tag="wk")
                    wv_sb = bulk.tile([128, EC, H], bf16, tag="wv")
                    nc.sync.dma_start(
                        wq_sb[:], wqT.rearrange("(c p) o -> p c o", p=128)
                    )
                    nc.sync.dma_start(
                        wk_sb[:], wkT.rearrange("(c p) o -> p c o", p=128)
                    )
                    nc.sync.dma_start(
                        wv_sb[:], wvT.rearrange("(c p) o -> p c o", p=128)
                    )
                    nc.vector.memset(v_sb[:, :, :, HD : HD + 1], 1.0)
                    with tc.tile_pool(name="xtp", bufs=3) as xtp, \
                         tc.tile_pool(name="ptp", bufs=6) as ptp, \
                         tc.tile_pool(name="rdp", bufs=3) as rdp, \
                         tc.tile_pool(name="rddr", bufs=3, space="DRAM") as rddr, \
                         tc.tile_pool(name="psA", bufs=2, space="PSUM") as psA, \
                         tc.tile_pool(name="psK", bufs=2, space="PSUM") as psK, \
                         tc.tile_pool(name="psC", bufs=2, space="PSUM") as psC:
                        emit_qkv(
                            nc,
                            (xtp, psA, psK, wq_sb, wk_sb, wv_sb,
                             kt_sb, qt_sb, v_sb),
                        )
                        if stage == "proj":
                            nc.gpsimd.dma_start(out[0:128, :], qt_sb[:, :, 0:128])
                            nc.gpsimd.dma_start(
                                out[128:256, :], kt_sb[:, :, 0:128]
                            )
                        else:
                            emit_attention(
                                nc,
                                (ptp, rdp, rddr, psA, psC,
                                 kt_sb, qt_sb, v_sb),
                            )
                if stage == "attn":
                    nc.gpsimd.dma_start(out[0:128, :], ctxT[:, :, 0:128])
                if stage == "full":
                    emit_tail(nc, tc)

    nc.compile()
    _BUILD_CACHE[key] = nc
    return nc


def make_in_maps(inputs, S_=S, SQ_=SQ):
    """Host-side sharding: slice/transpose/cast the full inputs into the 8
    per-core input maps."""
    bf16 = ml_dtypes.bfloat16
    hs = np.ascontiguousarray(np.asarray(inputs["hidden_states"], np.float32))
    am = np.asarray(inputs["attention_mask"], np.float32)
    q_w = np.asarray(inputs["q_w"], np.float32)
    k_w = np.asarray(inputs["k_w"], np.float32)
    v_w = np.asarray(inputs["v_w"], np.float32)
    o_w = np.asarray(inputs["o_w"], np.float32)
    q_b = np.asarray(inputs["q_b"], np.float32)
    k_b = np.asarray(inputs["k_b"], np.float32)
    v_b = np.asarray(inputs["v_b"], np.float32)
    o_b = np.asarray(inputs["o_b"], np.float32)
    ln_g = np.asarray(inputs["ln_g"], np.float32)
    ln_b = np.asarray(inputs["ln_b"], np.float32)

    scale = 1.0 / np.sqrt(HD)
    wqT_a = np.ascontiguousarray((q_w.T * scale).astype(bf16))
    wkT_a = np.ascontiguousarray(k_w.T.astype(bf16))
    wvT_a = np.ascontiguousarray(v_w.T.astype(bf16))
    woT_a = np.ascontiguousarray(o_w.T.astype(bf16))
    qbs = (q_b * scale).astype(np.float32)

    nb = hs.shape[0]
    xT_full = [np.ascontiguousarray(hs[b].T.astype(bf16)) for b in range(nb)]
    groups = NCORES // nb  # query-parallel cores per batch

    in_maps = []
    for c in range(NCORES):
        b, j = c // groups, c % groups
        sl = slice(j * SQ_, (j + 1) * SQ_)
        in_maps.append(
            {
                "xT": xT_full[b],
                "xTq": np.ascontiguousarray(xT_full[b][:, sl]),
                "wqT": wqT_a, "wkT": wkT_a, "wvT": wvT_a, "woT": woT_a,
                "qb": qbs, "kb": k_b, "vb": v_b,
                "mask": np.ascontiguousarray(am[b, 0, 0]),
                "xres": np.ascontiguousarray(hs[b, sl] + o_b[None, :]),
                "lng": ln_g, "lnb": ln_b,
            }
        )
    return in_maps


def run_cores(inputs, trace=False, **kwargs):
    from concourse.bass_utils import run_bass_kernel_spmd

    nc = build()
    in_maps = make_in_maps(inputs)
    res = run_bass_kernel_spmd(
        nc, in_maps, core_ids=list(range(NCORES)), trace=trace, **kwargs
    )
    nb = np.asarray(inputs["hidden_states"]).shape[0]
    groups = NCORES // nb
    out = np.empty((nb, S, H), np.float32)
    for c in range(NCORES):
        b, j = c // groups, c % groups
        out[b, j * SQ : (j + 1) * SQ] = res.results[c]["out"]
    return out, res


def kernel(**inputs):
    out, _ = run_cores(inputs, trace=False)
    return out


# revision 20
# speedup vs baseline: 115.6459x; 1.0548x over previous
"""Trainium2 Bass kernel for a BERT-style self-attention block (B=2, S=4096,
H=768, NH=12) sharded over 8 NeuronCores.

Sharding: data-parallel over batch (2) x query-block parallel (4) = 8 cores.
Each core computes K/V for the full sequence of its batch (replicated within
the 4-core group) and a disjoint 1024-query slice of the output, so no
collectives are needed and the LayerNorm epilogue is fully local.

Per-core dataflow (all matmuls bf16, fp32 accumulate):
  xT [H,S]  --PE-->  K^T [H,S], V [S,H] (+bias), Q^T [H,SQ] (pre-scaled 1/8)
  scores^T [skey,q] = K_h^T.T @ Q_h^T   (two heads row-packed per PE pass,
                                         each into its OWN psum bank)
  P^T = exp(scores^T + mask[skey])      (ScalarE, mask is per-partition bias)
  ctx_u^T [65,q] = [V_h | 1].T @ P^T    (ones column -> row 64 = softmax denom)
  ctx^T = ctx_u^T * recip(denom)        (partition-broadcast via DRAM bounce)
  out[s,:] = LN(ctx @ o_w.T + o_b + x)  (residual folded into xres on host)

All compute phases share one 8-bank PSUM budget and live in concurrently-open
pools so the Tile scheduler can overlap the projections with early attention
pairs; the bulk SBUF pool (K^T/V/Q^T/weights) closes before the LN tail opens
so the tail working set fits under the 208KB/partition cap.
"""

import numpy as np
import ml_dtypes

B, S, H, NH = 2, 4096, 768, 12
HD = H // NH  # 64
NCORES = 8
SQ = S // 4  # 1024 queries per core
LN_EPS = 1e-12

_BUILD_CACHE = {}


def build(S_=S, SQ_=SQ, stage="full", repeat=1, psk=2, psc=2, ptb=6):
    key = (S_, SQ_, stage, repeat, psk, psc, ptb)
    if key in _BUILD_CACHE:
        return _BUILD_CACHE[key]

    import concourse.mybir as mybir
    import concourse.tile as tile
    from concourse import bacc

    dt = mybir.dt
    f32, bf16 = dt.float32, dt.bfloat16
    AF = mybir.ActivationFunctionType
    OP = mybir.AluOpType

    EC = H // 128           # 6 contraction chunks over H
    OC = H // 128           # 6 output-channel chunks over H
    NSC512 = S_ // 512      # 512-wide s chunks (projection phase)
    NST = S_ // 128         # 128-wide s tiles (attention skey chunks)
    QT = min(512, SQ_)      # q tile for attention
    NQT = SQ_ // QT
    QCH = min(512, SQ_)     # q chunk for Q projection
    NQCH = SQ_ // QCH
    NP2 = NH // 2           # head pairs
    NSTQ = SQ_ // 128       # output s tiles

    nc = bacc.Bacc("TRN2", target_bir_lowering=False, debug=False)

    xT = nc.dram_tensor("xT", [H, S_], bf16, kind="ExternalInput")
    xTq = nc.dram_tensor("xTq", [H, SQ_], bf16, kind="ExternalInput")
    wqT = nc.dram_tensor("wqT", [H, H], bf16, kind="ExternalInput")
    wkT = nc.dram_tensor("wkT", [H, H], bf16, kind="ExternalInput")
    wvT = nc.dram_tensor("wvT", [H, H], bf16, kind="ExternalInput")
    woT = nc.dram_tensor("woT", [H, H], bf16, kind="ExternalInput")
    qb = nc.dram_tensor("qb", [H], f32, kind="ExternalInput")
    kb = nc.dram_tensor("kb", [H], f32, kind="ExternalInput")
    vb = nc.dram_tensor("vb", [H], f32, kind="ExternalInput")
    mask = nc.dram_tensor("mask", [S_], f32, kind="ExternalInput")
    xres = nc.dram_tensor("xres", [SQ_, H], f32, kind="ExternalInput")
    lng = nc.dram_tensor("lng", [H], f32, kind="ExternalInput")
    lnb = nc.dram_tensor("lnb", [H], f32, kind="ExternalInput")
    out = nc.dram_tensor("out", [SQ_, H], f32, kind="ExternalOutput")

    def emit_qkv(nc, pools):
        (xtp, psA, psK, wq_sb, wk_sb, wv_sb, kt_sb, qt_sb, v_sb) = pools
        # Q projection first: small, unblocks attention early
        xTq_r = xTq.rearrange("(c p) s -> p c s", p=128)
        for qc in range(NQCH):
            xtq = xtp.tile([128, EC, 512], bf16, tag="xt")
            nc.sync.dma_start(
                xtq[:, :, 0:QCH], xTq_r[:, :, qc * QCH : (qc + 1) * QCH]
            )
            for oc in range(OC):
                pq = psK.tile([128, 512], f32, tag="pk")
                for ec in range(EC):
                    nc.tensor.matmul(
                        pq[:, 0:QCH],
                        wq_sb[:, ec, oc * 128 : (oc + 1) * 128],
                        xtq[:, ec, 0:QCH],
                        start=(ec == 0), stop=(ec == EC - 1),
                    )
                nc.vector.tensor_scalar_add(
                    qt_sb[:, oc, qc * QCH : (qc + 1) * QCH],
                    pq[:, 0:QCH],
                    qb_sb[:, oc : oc + 1],
                )
        # K^T and V over the full sequence
        xT_r = xT.rearrange("(c p) s -> p c s", p=128)
        for sc in range(NSC512):
            xt = xtp.tile([128, EC, 512], bf16, tag="xt")
            nc.sync.dma_start(xt[:], xT_r[:, :, sc * 512 : (sc + 1) * 512])
            for oc in range(OC):
                pk = psK.tile([128, 512], f32, tag="pk")
                for ec in range(EC):
                    nc.tensor.matmul(
                        pk[:],
                        wk_sb[:, ec, oc * 128 : (oc + 1) * 128],
                        xt[:, ec],
                        start=(ec == 0), stop=(ec == EC - 1),
                    )
                nc.vector.tensor_scalar_add(
                    kt_sb[:, oc, sc * 512 : (sc + 1) * 512],
                    pk[:],
                    kb_sb[:, oc : oc + 1],
                )
            for t4 in range(4):
                st = sc * 4 + t4
                pva = psK.tile([128, 512], f32, tag="pk")
                pvb = psK.tile([128, 512], f32, tag="pk")
                for ec in range(EC):
                    xs = xt[:, ec, t4 * 128 : (t4 + 1) * 128]
                    nc.tensor.matmul(
                        pva[:], xs, wv_sb[:, ec, 0:512],
                        start=(ec == 0), stop=(ec == EC - 1),
                    )
                    nc.tensor.matmul(
                        pvb[:, 0 : H - 512], xs, wv_sb[:, ec, 512:H],
                        start=(ec == 0), stop=(ec == EC - 1),
                    )
                nc.vector.tensor_tensor(
                    v_sb[:, st, 0:8, 0:HD],
                    pva.rearrange("p (h d) -> p h d", d=HD),
                    vb_bc[:, 0:512].rearrange("p (h d) -> p h d", d=HD),
                    OP.add,
                )
                nc.vector.tensor_tensor(
                    v_sb[:, st, 8:NH, 0:HD],
                    pvb[:, 0 : H - 512].rearrange("p (h d) -> p h d", d=HD),
                    vb_bc[:, 512:H].rearrange("p (h d) -> p h d", d=HD),
                    OP.add,
                )

    def emit_attention(nc, pools):
        (ptp, rdp, rddr, psA, psC, kt_sb, qt_sb, v_sb) = pools
        do_ctx = stage not in ("attn1",)
        do_norm = stage not in ("attn1", "attn2")
        for qt in range(NQT):
            q0 = qt * QT
            for p2 in range(NP2):
                cx = [
                    psC.tile([HD + 1, QT], f32, tag="ctx", name=f"ctx{i}")
                    for i in range(2)
                ]
                for sc in range(NST):
                    sp = psA.tile([128, 1024], f32, tag="big")
                    nc.tensor.matmul(
                        sp[:, 0:QT],
                        kt_sb[0:64, p2, sc * 128 : (sc + 1) * 128],
                        qt_sb[0:64, p2, q0 : q0 + QT],
                        start=True, stop=True,
                    )
                    nc.tensor.matmul(
                        sp[:, 512 : 512 + QT],
                        kt_sb[64:128, p2, sc * 128 : (sc + 1) * 128],
                        qt_sb[64:128, p2, q0 : q0 + QT],
                        start=True, stop=True,
                    )
                    pt = ptp.tile([128, 2, QT], bf16, tag="pt")
                    nc.scalar.activation(
                        pt[:],
                        sp.rearrange("p (a q) -> p a q", a=2)[:, :, 0:QT],
                        AF.Exp,
                        bias=mask_sb[:, sc : sc + 1], scale=1.0,
                    )
                    if stage == "attn1":
                        nc.gpsimd.dma_start(
                            out[0:128, 0 : 2 * QT],
                            pt.rearrange("p a q -> p (a q)"),
                        )
                    if do_ctx:
                        nc.tensor.matmul(
                            cx[0][:], v_sb[:, sc, 2 * p2, :], pt[:, 0, :],
                            start=(sc == 0), stop=(sc == NST - 1),
                        )
                        nc.tensor.matmul(
                            cx[1][:], v_sb[:, sc, 2 * p2 + 1, :], pt[:, 1, :],
                            start=(sc == 0), stop=(sc == NST - 1),
                        )
                if stage == "attn2":
                    dbg = rdp.tile([65, QT], f32, tag="dbg")
                    nc.vector.tensor_copy(dbg[:], cx[0][:])
                    nc.gpsimd.dma_start(out[0:65, 0:QT], dbg[:])
                for i01 in range(2) if do_norm else []:
                    h = 2 * p2 + i01
                    rd = rdp.tile([1, QT], f32, tag="rd")
                    nc.vector.reciprocal(rd[:], cx[i01][HD : HD + 1, :])
                    rdd = rddr.tile([1, QT], f32, tag="rdd")
                    nc.sync.dma_start(rdd[:], rd[:])
                    rdb = rdp.tile([64, QT], f32, tag="rdb")
                    nc.sync.dma_start(rdb[:], rdd[:].to_broadcast((64, QT)))
                    nc.vector.tensor_tensor(
                        ctxT[
                            (h % 2) * 64 : (h % 2) * 64 + 64,
                            h // 2,
                            q0 : q0 + QT,
                        ],
                        cx[i01][0:HD, :],
                        rdb[:],
                        OP.mult,
                    )

    def emit_tail(nc, tc):
        with tc.tile_pool(name="tailc", bufs=1) as tpc, \
             tc.tile_pool(name="tailw", bufs=3) as tpw, \
             tc.tile_pool(name="ys", bufs=NSTQ) as yp, \
             tc.tile_pool(name="ps3", bufs=2, space="PSUM") as ps3:
            wo_sb = tpc.tile([128, EC, H], bf16, tag="wo")
            lng_bc = tpc.tile([128, H], f32, tag="lngbc")
            lnb_bc = tpc.tile([128, H], f32, tag="lnbbc")
            eps_ap = tpc.tile([128, 1], f32, tag="eps")
            ss_all = tpc.tile([128, NSTQ], f32, tag="ss")
            negmu_all = tpc.tile([128, NSTQ], f32, tag="negmu")
            std_all = tpc.tile([128, NSTQ], f32, tag="std")
            rstd_all = tpc.tile([128, NSTQ], f32, tag="rstd")
            nc.sync.dma_start(wo_sb[:], woT.rearrange("(c p) o -> p c o", p=128))
            nc.sync.dma_start(lng_bc[:], lng[None, :].to_broadcast((128, H)))
            nc.sync.dma_start(lnb_bc[:], lnb[None, :].to_broadcast((128, H)))
            nc.vector.memset(eps_ap[:], float(LN_EPS))
            ys = []
            for st in range(NSTQ):
                po = ps3.tile([128, H], f32, tag="po")
                for dc in range(EC):
                    lh = ctxT[:, dc, st * 128 : (st + 1) * 128]
                    nc.tensor.matmul(
                        po[:, 0:512], lh, wo_sb[:, dc, 0:512],
                        start=(dc == 0), stop=(dc == EC - 1),
                    )
                    nc.tensor.matmul(
                        po[:, 512:H], lh, wo_sb[:, dc, 512:H],
                        start=(dc == 0), stop=(dc == EC - 1),
                    )
                xr = tpw.tile([128, H], f32, tag="xr")
                nc.sync.dma_start(xr[:], xres[st * 128 : (st + 1) * 128, :])
                y = yp.tile([128, H], f32, tag="y")
                ysum = tpw.tile([128, 1], f32, tag="ysum")
                nc.vector.tensor_tensor(y[:], po[:], xr[:], OP.add)
                nc.vector.reduce_sum(ysum[:], y[:], axis=mybir.AxisListType.X)
                nc.vector.tensor_scalar_mul(
                    negmu_all[:, st : st + 1], ysum[:], -1.0 / H
                )
                sq = tpw.tile([128, H], f32, tag="scratch")
                nc.scalar.activation(
                    sq[:], y[:], AF.Square,
                    bias=negmu_all[:, st : st + 1], scale=1.0,
                    accum_out=ss_all[:, st : st + 1],
                )
                ys.append(y)
            nc.scalar.activation(
                std_all[:], ss_all[:], AF.Sqrt,
                bias=eps_ap[:, 0:1], scale=1.0 / H,
            )
            nc.vector.reciprocal(rstd_all[:], std_all[:])
            for st in range(NSTQ):
                t1 = tpw.tile([128, H], f32, tag="scratch")
                nc.vector.tensor_scalar(
                    t1[:], ys[st][:],
                    negmu_all[:, st : st + 1], rstd_all[:, st : st + 1],
                    OP.add, OP.mult,
                )
                t2 = tpw.tile([128, H], f32, tag="scratch")
                nc.vector.tensor_tensor(t2[:], t1[:], lng_bc[:], OP.mult)
                ot = tpw.tile([128, H], f32, tag="scratch")
                nc.vector.tensor_tensor(ot[:], t2[:], lnb_bc[:], OP.add)
                nc.sync.dma_start(out[st * 128 : (st + 1) * 128, :], ot[:])

    with tile.TileContext(nc) as tc:
        with tc.tile_pool(name="persist", bufs=1) as pp:
            ctxT = pp.tile([128, EC, SQ_], bf16, tag="ctxT")
            qb_sb = pp.tile([128, OC], f32, tag="qb")
            kb_sb = pp.tile([128, OC], f32, tag="kb")
            mask_sb = pp.tile([128, NST], f32, tag="mask")
            vb_bc = pp.tile([128, H], f32, tag="vbbc")
            nc.sync.dma_start(qb_sb[:], qb.rearrange("(c p) -> p c", p=128))
            nc.sync.dma_start(kb_sb[:], kb.rearrange("(c p) -> p c", p=128))
            nc.sync.dma_start(mask_sb[:], mask.rearrange("(c p) -> p c", p=128))
            nc.sync.dma_start(vb_bc[:], vb[None, :].to_broadcast((128, H)))

            for rep_ in range(repeat):
                with tc.tile_pool(name="bulk", bufs=1) as bulk:
                    kt_sb = bulk.tile([128, OC, S_], bf16, tag="kt")
                    qt_sb = bulk.tile([128, OC, SQ_], bf16, tag="qt")
                    v_sb = bulk.tile([128, NST, NH, HD + 1], bf16, tag="v")
                    wq_sb = bulk.tile([128, EC, H], bf16, tag="wq")
                    wk_sb = bulk.tile([128, EC, H], bf16, tag="wk")
                    wv_sb = bulk.tile([128, EC, H], bf16, tag="wv")
                    nc.sync.dma_start(
                        wq_sb[:], wqT.rearrange("(c p) o -> p c o", p=128)
                    )
                    nc.sync.dma_start(
                        wk_sb[:], wkT.rearrange("(c p) o -> p c o", p=128)
                    )
                    nc.sync.dma_start(
                        wv_sb[:], wvT.rearrange("(c p) o -> p c o", p=128)
                    )
                    nc.vector.memset(v_sb[:, :, :, HD : HD + 1], 1.0)
                    with tc.tile_pool(name="xtp", bufs=3) as xtp, \
                         tc.tile_pool(name="ptp", bufs=ptb) as ptp, \
                         tc.tile_pool(name="rdp", bufs=3) as rdp, \
                         tc.tile_pool(name="rddr", bufs=3, space="DRAM") as rddr, \
                         tc.tile_pool(name="psA", bufs=2, space="PSUM") as psA, \
                         tc.tile_pool(name="psK", bufs=psk, space="PSUM") as psK, \
                         tc.tile_pool(name="psC", bufs=psc, space="PSUM") as psC:
                        emit_qkv(
                            nc,
                            (xtp, psA, psK, wq_sb, wk_sb, wv_sb,
                             kt_sb, qt_sb, v_sb),
                        )
                        if stage == "proj":
                            nc.gpsimd.dma_start(out[0:128, :], qt_sb[:, :, 0:128])
                            nc.gpsimd.dma_start(
                                out[128:256, :], kt_sb[:, :, 0:128]
                            )
                        else:
                            emit_attention(
                                nc,
                                (ptp, rdp, rddr, psA, psC,
                                 kt_sb, qt_sb, v_sb),
                            )
                if stage == "attn":
                    nc.gpsimd.dma_start(out[0:128, :], ctxT[:, :, 0:128])
                if stage == "full":
                    emit_tail(nc, tc)

    nc.compile()
    _BUILD_CACHE[key] = nc
    return nc


def make_in_maps(inputs, S_=S, SQ_=SQ):
    """Host-side sharding: slice/transpose/cast the full inputs into the 8
    per-core input maps."""
    bf16 = ml_dtypes.bfloat16
    hs = np.ascontiguousarray(np.asarray(inputs["hidden_states"], np.float32))
    am = np.asarray(inputs["attention_mask"], np.float32)
    q_w = np.asarray(inputs["q_w"], np.float32)
    k_w = np.asarray(inputs["k_w"], np.float32)
    v_w = np.asarray(inputs["v_w"], np.float32)
    o_w = np.asarray(inputs["o_w"], np.float32)
    q_b = np.asarray(inputs["q_b"], np.float32)
    k_b = np.asarray(inputs["k_b"], np.float32)
    v_b = np.asarray(inputs["v_b"], np.float32)
    o_b = np.asarray(inputs["o_b"], np.float32)
    ln_g = np.asarray(inputs["ln_g"], np.float32)
    ln_b = np.asarray(inputs["ln_b"], np.float32)

    scale = 1.0 / np.sqrt(HD)
    wqT_a = np.ascontiguousarray((q_w.T * scale).astype(bf16))
    wkT_a = np.ascontiguousarray(k_w.T.astype(bf16))
    wvT_a = np.ascontiguousarray(v_w.T.astype(bf16))
    woT_a = np.ascontiguousarray(o_w.T.astype(bf16))
    qbs = (q_b * scale).astype(np.float32)

    nb = hs.shape[0]
    xT_full = [np.ascontiguousarray(hs[b].T.astype(bf16)) for b in range(nb)]
    groups = NCORES // nb  # query-parallel cores per batch

    in_maps = []
    for c in range(NCORES):
        b, j = c // groups, c % groups
        sl = slice(j * SQ_, (j + 1) * SQ_)
        in_maps.append(
            {
                "xT": xT_full[b],
                "xTq": np.ascontiguousarray(xT_full[b][:, sl]),
                "wqT": wqT_a, "wkT": wkT_a, "wvT": wvT_a, "woT": woT_a,
                "qb": qbs, "kb": k_b, "vb": v_b,
                "mask": np.ascontiguousarray(am[b, 0, 0]),
                "xres": np.ascontiguousarray(hs[b, sl] + o_b[None, :]),
                "lng": ln_g, "lnb": ln_b,
            }
        )
    return in_maps


def run_cores(inputs, trace=False, **kwargs):
    from concourse.bass_utils import run_bass_kernel_spmd

    nc = build()
    in_maps = make_in_maps(inputs)
    res = run_bass_kernel_spmd(
        nc, in_maps, core_ids=list(range(NCORES)), trace=trace, **kwargs
    )
    nb = np.asarray(inputs["hidden_states"]).shape[0]
    groups = NCORES // nb
    out = np.empty((nb, S, H), np.float32)
    for c in range(NCORES):
        b, j = c // groups, c % groups
        out[b, j * SQ : (j + 1) * SQ] = res.results[c]["out"]
    return out, res


def kernel(**inputs):
    out, _ = run_cores(inputs, trace=False)
    return out
